# revision 24
# baseline (speedup 1.0000x reference)
"""MoE routing kernel for Trainium2 (8 NeuronCores, batch-parallel).

Problem: nn_MoE_47278999994656.
  x [8, 256, 80, 80] f32 + gate Linear(256->5) + 5 experts
  (residual conv1x1 on each 128-ch half, gated by a sigmoid transform),
  top-1 masked-softmax gate => weights are EXACTLY one-hot, so
  out[b] = expert_{argmax_e logits[b,e]}(x[b]).

Sharding: data-parallel over batch, core i computes batch item i.
Per core: x is DMA'd once with an inline f32->bf16 cast; the gate runs as
PSUM-accumulated bf16 matmuls (top-2 logit-gap margin is ~80x the bf16
noise); the selected expert's weights are materialized by a mask-weighted
sum over the 5 experts; the expert itself runs as bf16 matmuls with
fused residual (I+W), a partition-stacked H layer, and a replicated-Wt2
A-matmul that broadcasts the sigmoid argument to all 128 partitions.
"""

import numpy as np

import concourse.bacc as bacc_mod
import concourse.bass as bass
import concourse.mybir as mybir
import concourse.tile as tile
from concourse.bass import ts
from concourse.bass_utils import run_bass_kernel_spmd

B, C, H, W = 8, 256, 80, 80
HW = H * W          # 6400
HALF = 128
QUARTER = 64
E = 5
NCORES = 8

# expert-layer chunks: 12 x 512 + 1 x 256 (psum bank holds 512 f32)
CHUNKS = [(i * 512, 512) for i in range(12)] + [(6144, 256)]
BLOCKS = [CHUNKS[0:4], CHUNKS[4:8], CHUNKS[8:13]]
DMACH = 1600        # input DMA chunk columns
NDMA = HW // DMACH  # 4
GCH = 512           # gate matmul chunk

# U_all free-dim layout (per expert, partition dim = 128):
#   [0:128)    (I + Wrgb)^T        [c, o]
#   [128:256)  (I + Wtir)^T        [c, o]
#   [256:320)  Wt1^T               [o, m]   (m = 64)
#   [320:448)  Wt2 replicated      [m, :]   rows 0:64 and 64:128 both = rep
UF = 448
U_RGB = 0
U_TIR = 128
U_WT1 = 256
U_WT2 = 320

F32 = mybir.dt.float32
BF16 = mybir.dt.bfloat16


def build_nc() -> bass.Bass:
    nc = bacc_mod.Bacc()

    x_d = nc.dram_tensor("x", [C, HW], F32, kind="ExternalInput")
    u_d = nc.dram_tensor("u", [HALF, E, UF], BF16, kind="ExternalInput")
    bias_d = nc.dram_tensor("bias", [HALF, E, 4], F32, kind="ExternalInput")
    wg_d = nc.dram_tensor("wg", [HALF, 2, E], BF16, kind="ExternalInput")
    bg_d = nc.dram_tensor("bg", [1, E], F32, kind="ExternalInput")
    out_d = nc.dram_tensor("out", [HALF, HW], F32, kind="ExternalOutput")

    with tile.TileContext(nc) as tc:
        with (
            tc.tile_pool(name="big", bufs=1) as big,
            tc.tile_pool(name="const", bufs=1) as const,
            tc.tile_pool(name="small", bufs=1) as small,
            tc.tile_pool(name="hpool", bufs=6) as hpool,
            tc.tile_pool(name="ppool", bufs=4) as ppool,
            tc.tile_pool(name="gps", bufs=1, space="PSUM") as gps,
            tc.tile_pool(name="dps_p", bufs=2, space="PSUM") as dps_p,
            tc.tile_pool(name="hps_p", bufs=2, space="PSUM") as hps_p,
            tc.tile_pool(name="aps_p", bufs=3, space="PSUM") as aps_p,
        ):
            # ---- persistent SBUF ----
            xs = big.tile([HALF, 2, HW], F32)        # 51.2 KB/part
            xb = big.tile([HALF, 2, HW], BF16)       # 25.6 KB/part
            dsb = big.tile([HALF, 2, HW], BF16)      # 25.6 KB/part
            ssb_t = big.tile([HALF, 2, HW], BF16)    # 25.6 KB/part
            u_all = const.tile([HALF, E, UF], BF16)  # 4.5 KB/part
            bias_all = const.tile([HALF, E, 4], F32)
            wg = const.tile([HALF, 2, E], BF16)
            bgx = const.tile([1, E], F32)

            nc.scalar.dma_start(out=u_all[:], in_=u_d[:])
            nc.scalar.dma_start(out=bias_all[:], in_=bias_d[:])
            nc.scalar.dma_start(out=wg[:], in_=wg_d[:])
            nc.scalar.dma_start(out=bgx[:], in_=bg_d[:])

            # ---- phase 1: HWDGE x DMA + ACT f32->bf16 casts ----
            for h in range(2):
                for j in range(NDMA):
                    sl = ts(j, DMACH)
                    nc.sync.dma_start(
                        out=xs[:, h, sl], in_=x_d[h * HALF : (h + 1) * HALF, sl]
                    )
            for h in range(2):
                for j in range(NDMA):
                    sl = ts(j, DMACH)
                    nc.vector.tensor_copy(xb[:, h, sl], xs[:, h, sl])

            # gate: Y[5, 512] += WgT_half^T @ xb chunks (PSUM accumulate)
            yg = gps.tile([E, GCH], F32, tag="g")
            gsl = []
            for h in range(2):
                o = 0
                while o < HW:
                    n = min(GCH, HW - o)
                    gsl.append((h, o, n))
                    o += n
            for k, (h, o, n) in enumerate(gsl):
                nc.tensor.matmul(
                    yg[:, 0:n],
                    lhsT=wg[:, h, :],
                    rhs=xb[:, h, o : o + n],
                    start=(k == 0),
                    stop=(k == len(gsl) - 1),
                )

            l51 = small.tile([E, 1], F32)
            nc.vector.reduce_sum(l51, yg, axis=mybir.AxisListType.X)
            t32a = small.tile([32, 32], F32)
            t32b = small.tile([32, 32], F32)
            nc.vector.memset(t32a, 0.0)
            nc.vector.tensor_copy(t32a[0:E, 0:1], l51)
            nc.vector.transpose(t32b, t32a)
            lrow = small.tile([1, E], F32)
            nc.vector.tensor_add(lrow, t32b[0:1, 0:E], bgx[0:1, :])
            lmax = small.tile([1, 1], F32)
            nc.vector.reduce_max(lmax, lrow, axis=mybir.AxisListType.X)
            mrow = small.tile([1, E], F32)
            nc.vector.tensor_scalar(
                out=mrow, in0=lrow, scalar1=lmax, scalar2=None,
                op0=mybir.AluOpType.is_equal,
            )
            ones1 = small.tile([1, HALF], F32)
            nc.vector.memset(ones1, 1.0)
            mps = gps.tile([HALF, E], F32, tag="g")
            nc.tensor.matmul(mps, lhsT=ones1, rhs=mrow)
            mbc = small.tile([HALF, E], F32)
            nc.vector.tensor_copy(mbc, mps)

            # warmup matmuls: keep the PE HAM busy up to the select phase
            junk = gps.tile([HALF, 512], F32, tag="g")
            for wj in range(6):
                off = 512 * (wj % 4)
                nc.tensor.matmul(
                    junk, lhsT=u_all[:, wj % E, 0:HALF],
                    rhs=xb[:, 0, off : off + 512],
                )

            # ---- select expert weights (mask is exactly one-hot) ----
            # junk matmuls chained on each select step keep the PE warm
            # through the serial select so phase 2 starts at full clock.
            usel = small.tile([HALF, UF], BF16)
            for lo, hi in ((0, 2 * HALF), (2 * HALF, UF)):
                nc.vector.tensor_scalar_mul(
                    usel[:, lo:hi], u_all[:, 0, lo:hi], mbc[:, 0:1]
                )
                for e in range(1, E):
                    utmp = hpool.tile([HALF, 2 * HALF], BF16, tag="utmp")
                    w = hi - lo
                    nc.vector.tensor_scalar_mul(
                        utmp[:, 0:w], u_all[:, e, lo:hi], mbc[:, e : e + 1]
                    )
                    nc.vector.tensor_add(
                        usel[:, lo:hi], usel[:, lo:hi], utmp[:, 0:w]
                    )
                    if lo == 0 and e % 2 == 1:
                        nc.tensor.matmul(
                            junk, lhsT=utmp[:, 0:HALF], rhs=xb[:, 1, 0:512]
                        )
            bsel = small.tile([HALF, 4], F32)
            btmp = small.tile([HALF, 4], F32)
            nc.vector.tensor_scalar_mul(bsel, bias_all[:, 0, :], mbc[:, 0:1])
            for e in range(1, E):
                nc.vector.tensor_scalar_mul(btmp, bias_all[:, e, :], mbc[:, e : e + 1])
                nc.vector.tensor_add(bsel, bsel, btmp)
            bscr = small.tile([HALF, 4], F32)
            nc.scalar.copy(bscr, bsel)               # ACT observes DVE(bsel)

            # ---- phase 2: selected expert, chunk-major software pipeline ----
            pend = []   # chunks combined but not yet stored
            for ci, (off, n) in enumerate(CHUNKS):
                # D layer
                dr = dps_p.tile([HALF, 512], F32, tag="dps")
                nc.tensor.matmul(
                    dr[:, 0:n], lhsT=usel[:, 0:HALF], rhs=xb[:, 0, off : off + n]
                )
                nc.vector.tensor_scalar_add(
                    dsb[:, 0, off : off + n], dr[:, 0:n], bsel[:, 0:1]
                )
                dt = dps_p.tile([HALF, 512], F32, tag="dps")
                nc.tensor.matmul(
                    dt[:, 0:n], lhsT=usel[:, HALF : 2 * HALF],
                    rhs=xb[:, 1, off : off + n],
                )
                nc.scalar.activation(
                    out=dsb[:, 1, off : off + n], in_=dt[:, 0:n],
                    func=mybir.ActivationFunctionType.Identity,
                    bias=bsel[:, 1:2],
                )
                # H layer (stacked halves)
                hps = hps_p.tile([HALF, 512], F32, tag="hps")
                nc.tensor.matmul(
                    hps[0:QUARTER, 0:n],
                    lhsT=usel[:, U_WT1 : U_WT1 + QUARTER],
                    rhs=dsb[:, 0, off : off + n],
                )
                nc.tensor.matmul(
                    hps[QUARTER:HALF, 0:n],
                    lhsT=usel[:, U_WT1 : U_WT1 + QUARTER],
                    rhs=dsb[:, 1, off : off + n],
                    tile_position=(0, QUARTER),
                )
                hsb = hpool.tile([HALF, 512], BF16, tag="hsb")
                if ci % 2 == 0:
                    nc.vector.tensor_scalar(
                        out=hsb[:, 0:n], in0=hps[:, 0:n],
                        scalar1=bsel[:, 2:3], scalar2=0.0,
                        op0=mybir.AluOpType.add, op1=mybir.AluOpType.max,
                    )
                else:
                    nc.scalar.activation(
                        out=hsb[:, 0:n], in_=hps[:, 0:n],
                        func=mybir.ActivationFunctionType.Relu,
                        bias=bsel[:, 2:3],
                    )
                # A layer + sigmoid (broadcast S to all partitions)
                for s in range(2):
                    aps = aps_p.tile([HALF, 512], F32, tag="aps")
                    nc.tensor.matmul(
                        aps[:, 0:n],
                        lhsT=usel[
                            s * QUARTER : (s + 1) * QUARTER, U_WT2 : U_WT2 + HALF
                        ],
                        rhs=hsb[s * QUARTER : (s + 1) * QUARTER, 0:n],
                        tile_position=(s * QUARTER, 0),
                    )
                    nc.scalar.activation(
                        out=ssb_t[:, s, off : off + n], in_=aps[:, 0:n],
                        func=mybir.ActivationFunctionType.Sigmoid,
                        bias=bsel[:, 3:4],
                    )
                # combine
                prt = ppool.tile([HALF, 512], BF16, tag="prt")
                ob = ppool.tile([HALF, 512], BF16, tag="ob")
                nc.vector.tensor_mul(
                    prt[:, 0:n], dsb[:, 0, off : off + n], ssb_t[:, 0, off : off + n]
                )
                nc.vector.tensor_mul(
                    ob[:, 0:n], dsb[:, 1, off : off + n], ssb_t[:, 1, off : off + n]
                )
                nc.vector.tensor_add(ob[:, 0:n], ob[:, 0:n], prt[:, 0:n])
                pend.append((off, n, ob))
                if len(pend) == 2 or ci == len(CHUNKS) - 1:
                    for poff, pn, pob in pend:
                        nc.gpsimd.dma_start(
                            out=out_d[:, poff : poff + pn], in_=pob[:, 0:pn]
                        )
                    pend = []

    nc.compile()
    return nc


def _pack_inputs(x, Wg, bg, Wrgb, brgb, Wtir, btir, Wt1, bt1, Wt2, bt2):
    import ml_dtypes
    eye = np.eye(HALF, dtype=np.float32)
    u = np.zeros((E, HALF, UF), dtype=np.float32)
    for e in range(E):
        u[e, :, U_RGB : U_RGB + HALF] = Wrgb[e].T + eye
        u[e, :, U_TIR : U_TIR + HALF] = Wtir[e].T + eye
        u[e, :, U_WT1 : U_WT1 + QUARTER] = Wt1[e].T
        u[e, :, U_WT2 : U_WT2 + HALF] = np.tile(
            np.repeat(Wt2[e, 0][:, None], HALF, axis=1), (2, 1)
        )
    u = np.ascontiguousarray(u.transpose(1, 0, 2)).astype(ml_dtypes.bfloat16)

    bias = np.zeros((E, HALF, 4), dtype=np.float32)
    for e in range(E):
        bias[e, :, 0] = brgb[e]
        bias[e, :, 1] = btir[e]
        bias[e, 0:QUARTER, 2] = bt1[e]
        bias[e, QUARTER:HALF, 2] = bt1[e]
        bias[e, :, 3] = bt2[e, 0]
    bias = np.ascontiguousarray(bias.transpose(1, 0, 2))

    wgt = Wg.T.astype(np.float32)                   # [256, 5]
    wg_p = np.ascontiguousarray(
        np.stack([wgt[:HALF], wgt[HALF:]], axis=1)
    ).astype(ml_dtypes.bfloat16)                    # [128, 2, 5]
    bgx = np.ascontiguousarray((bg * float(HW))[None, :].astype(np.float32))

    common = {"u": u, "bias": bias, "wg": wg_p, "bg": bgx}
    in_maps = []
    for b in range(B):
        m = dict(common)
        m["x"] = np.ascontiguousarray(x[b].reshape(C, HW).astype(np.float32))
        in_maps.append(m)
    return in_maps


_NC_CACHE = {}


def _get_nc():
    if "nc" not in _NC_CACHE:
        _NC_CACHE["nc"] = build_nc()
    return _NC_CACHE["nc"]


def kernel(x, Wg, bg, Wrgb, brgb, Wtir, btir, Wt1, bt1, Wt2, bt2, **run_kw):
    nc = _get_nc()
    in_maps = _pack_inputs(
        np.asarray(x), np.asarray(Wg), np.asarray(bg), np.asarray(Wrgb),
        np.asarray(brgb), np.asarray(Wtir), np.asarray(btir),
        np.asarray(Wt1), np.asarray(bt1), np.asarray(Wt2), np.asarray(bt2),
    )
    res = run_bass_kernel_spmd(nc, in_maps, core_ids=list(range(NCORES)), **run_kw)
    out = np.stack([r["out"] for r in res.results], axis=0)  # [8, 128, 6400]
    if run_kw:
        kernel.last_results = res
    return out.reshape(B, HALF, H, W).astype(np.float32)
